# revision 26
# baseline (speedup 1.0000x reference)
"""GQA attention block (QKV proj + RoPE + KV cache append + softmax attention)
on 8 Trainium2 NeuronCores, tensor-parallel over heads.

Sharding: core c owns q-heads [4c, 4c+4) and kv-head c. Each core computes its
head slice over all tokens; host concatenates the per-core output columns.

start_pos is specialized to 0 (the cache is zero-filled and fully overwritten
by the current 2048 tokens, so keys/values == rope(x@wk), x@wv).

Schedule (v2): softmax exp() runs on the ACT engine at 1 elem/cycle/partition
and totals ~286us/core -- more than the attention-phase PE work -- so
attention chunks are interleaved with the batch-1 q-projection passes at
matmul granularity, hiding the exp under projection matmuls:
  region 1: k/v/q projections for batch-0 tokens (pc 0..3), rope epilogues
  region 2: k/v projections for batch-1 tokens (pc 4..7)
  region 3: 16 groups of [q-pass (pc,h)] x [2 attention chunks], with the
            q matmuls injected between score matmuls so PE keeps working
            while ACT drains the exp queue.
Output is written untransposed as [B, HPC, HD, S]; the host reassembles.
"""

import sys

sys.path.insert(0, "/opt/trn_rl_repo")

import ml_dtypes
import numpy as np

import concourse.bass as bass
import concourse.tile as tile
from concourse import bacc, mybir
from concourse.bass_utils import run_bass_kernel_spmd
from concourse.masks import make_identity

F32 = mybir.dt.float32
BF16 = mybir.dt.bfloat16

B, S, D = 2, 2048, 4096
HQ, HKV, HD = 32, 8, 128
NCORES = 8
HPC = HQ // NCORES          # q heads per core (4)
QDIM = HPC * HD             # per-core q output dim (512)
TOK = B * S                 # 4096 tokens across both batches
KCH = D // 128              # 32 contraction chunks of 128
PCH = 8                     # projection token chunks
PCW = TOK // PCH            # 512 tokens per chunk
SCH = 4                     # s-chunks per batch in attention
SCW = S // SCH              # 512
NTT = S // 128              # 16 key tiles per batch
SCALE = 1.0 / float(np.sqrt(HD))

LAST_EXEC_NS = None


def _build_program():
    nc = bacc.Bacc("TRN2", target_bir_lowering=False, debug=False,
                   num_devices=NCORES)

    xt = nc.declare_dram_parameter("xt", [D, TOK], BF16, isOutput=False)
    wq = nc.declare_dram_parameter("wq", [D, QDIM], BF16, isOutput=False)
    wk = nc.declare_dram_parameter("wk", [D, HD], BF16, isOutput=False)
    wv = nc.declare_dram_parameter("wv", [D, HD], BF16, isOutput=False)
    cc = nc.declare_dram_parameter("cc", [128, S], BF16, isOutput=False)
    ss = nc.declare_dram_parameter("ss", [128, S], BF16, isOutput=False)
    out = nc.declare_dram_parameter("out", [B, HPC, HD, S], F32,
                                    isOutput=True)

    with tile.TileContext(nc) as tc:
        pers_cm = tc.tile_pool(name="pers", bufs=1)
        pers = pers_cm.__enter__()

        ccs = pers.tile([128, S], BF16)
        sss = pers.tile([128, S], BF16)
        qT0 = pers.tile([128, HPC, S], BF16)     # batch-0 q, [d, head, tok]
        kTb = pers.tile([128, TOK], BF16)        # [d, tok]
        vtok = pers.tile([128, B * NTT, HD], BF16)  # [t, (b,tt), dv]
        ones128 = pers.tile([128, 128], BF16)

        # pool stack (LIFO close order): pers, wqp, xsp, ropep live through
        # region 3; wkv + pp1 close after region 2.
        wq_cm = tc.tile_pool(name="wqp", bufs=1)
        wqp = wq_cm.__enter__()
        xsp_cm = tc.tile_pool(name="xsp", bufs=36)
        xsp = xsp_cm.__enter__()
        ropep_cm = tc.tile_pool(name="ropep", bufs=2)
        ropep = ropep_cm.__enter__()
        wkv_cm = tc.tile_pool(name="wkv", bufs=1)
        wkv = wkv_cm.__enter__()

        wkb = wkv.tile([128, KCH, HD], BF16)
        wvb = wkv.tile([128, KCH, HD], BF16)
        vTb = wkv.tile([128, TOK], BF16)         # [dv, tok], regions 1-2
        id_bf = wkv.tile([128, 128], BF16)
        wqb = wqp.tile([128, KCH, QDIM], BF16)
        # weights all on the gpsimd queue in need-order (k+v pass first,
        # then q passes); the sync queue carries x exclusively so the
        # startup HBM demand stays under the per-core bandwidth.
        for kc in range(KCH):
            nc.gpsimd.dma_start(out=wkb[:, kc, :],
                                in_=wk[kc * 128:(kc + 1) * 128, :])
        for kc in range(KCH):
            nc.gpsimd.dma_start(out=wvb[:, kc, :],
                                in_=wv[kc * 128:(kc + 1) * 128, :])
        for kc in range(KCH):
            nc.gpsimd.dma_start(out=wqb[:, kc, :],
                                in_=wq[kc * 128:(kc + 1) * 128, :])
        nc.scalar.dma_start(out=ccs, in_=cc[:])
        nc.scalar.dma_start(out=sss, in_=ss[:])
        make_identity(nc, id_bf)
        nc.vector.memset(ones128, 1.0)

        def load_slab(pc):
            tiles = []
            for kc in range(KCH):
                xf = xsp.tile([128, PCW], BF16, tag="xs", name="xs")
                nc.sync.dma_start(
                    out=xf,
                    in_=xt[kc * 128:(kc + 1) * 128,
                           pc * PCW:(pc + 1) * PCW])
                tiles.append(xf)
            return tiles

        def rope_epilogue(ps, dst, cc_off):
            cc_sl = bass.ds(cc_off, PCW)
            t1 = ropep.tile([128, PCW], F32, tag="t1")
            t2 = ropep.tile([128, PCW], F32, tag="t2")
            swp = ropep.tile([128, PCW], F32, tag="swp")
            nc.vector.tensor_mul(t1, ps, ccs[:, cc_sl])
            # pair-partner swap: cross-partition-base copies
            nc.scalar.copy(swp[0:64], ps[64:128])
            nc.scalar.copy(swp[64:128], ps[0:64])
            nc.vector.tensor_mul(t2, swp, sss[:, cc_sl])
            nc.vector.tensor_add(dst, t1, t2)

        # ---------------- regions 1+2: projections ----------------
        # kc-major: each x tile is consumed by its 6 (or 2) matmuls
        # back-to-back, so its ring slot frees ~1.3us after the DMA and the
        # next slab streams in fully overlapped.
        pp1_cm = tc.tile_pool(name="pp1", bufs=6, space="PSUM")
        pp1 = pp1_cm.__enter__()

        def v_transposes_pc(pc):
            # 4 token blocks of this pc, spread per-pc so the PE/DVE work
            # never bunches at a region boundary
            for j in range(4):
                tt = pc * 4 + j
                pt = pp1.tile([128, 128], BF16, tag="vt", name="pt", bufs=2)
                nc.tensor.transpose(
                    pt, vTb[:, tt * 128:(tt + 1) * 128], id_bf)
                nc.vector.tensor_copy(vtok[:, tt, :], pt)

        def proj_pc(pc, with_q):
            tok_sl = bass.ds(pc * PCW, PCW)
            cc_off = (pc * PCW) % S
            nps = 2 + (HPC if with_q else 0)
            lhs = [wkb, wvb] + [wqb[:, :, h * HD:(h + 1) * HD]
                                for h in range(HPC)][:nps - 2]
            psums = [pp1.tile([128, PCW], F32, tag="proj", name="proj")
                     for _ in range(nps)]
            for kc in range(KCH):
                xf = xsp.tile([128, PCW], BF16, tag="xs", name="xs")
                nc.sync.dma_start(
                    out=xf, in_=xt[kc * 128:(kc + 1) * 128,
                                   pc * PCW:(pc + 1) * PCW])
                for ot in range(nps):
                    nc.tensor.matmul(psums[ot], lhs[ot][:, kc, :], xf,
                                     start=(kc == 0), stop=(kc == KCH - 1))
            rope_epilogue(psums[0], kTb[:, tok_sl], cc_off)
            nc.scalar.copy(vTb[:, tok_sl], psums[1])
            v_transposes_pc(pc)
            for h in range(nps - 2):
                rope_epilogue(psums[2 + h],
                              qT0[:, h, bass.ds(pc * PCW, PCW)], cc_off)

        # pc 0 runs as three PAIRED passes [k+v][q0+q1][q2+q3]: two matmuls
        # per x tile keeps the startup consumption rate (~300 GB/s) under
        # the x DMA stream, and the weight need-times spread out.
        slab0 = load_slab(0)
        tok_sl0 = bass.ds(0, PCW)

        def pass0_pair(lhsA, lhsB):
            psA = pp1.tile([128, PCW], F32, tag="proj", name="proj")
            psB = pp1.tile([128, PCW], F32, tag="proj", name="proj")
            for kc in range(KCH):
                nc.tensor.matmul(psA, lhsA[:, kc, :], slab0[kc],
                                 start=(kc == 0), stop=(kc == KCH - 1))
                nc.tensor.matmul(psB, lhsB[:, kc, :], slab0[kc],
                                 start=(kc == 0), stop=(kc == KCH - 1))
            return psA, psB

        psk0, psv0 = pass0_pair(wkb, wvb)
        rope_epilogue(psk0, kTb[:, tok_sl0], 0)
        nc.scalar.copy(vTb[:, tok_sl0], psv0)
        ps0, ps1 = pass0_pair(wqb[:, :, 0 * HD:1 * HD], wqb[:, :, 1 * HD:2 * HD])
        rope_epilogue(ps0, qT0[:, 0, tok_sl0], 0)
        rope_epilogue(ps1, qT0[:, 1, tok_sl0], 0)
        v_transposes_pc(0)   # v-copy has long finished by now
        ps2, ps3 = pass0_pair(wqb[:, :, 2 * HD:3 * HD], wqb[:, :, 3 * HD:4 * HD])
        rope_epilogue(ps2, qT0[:, 2, tok_sl0], 0)
        rope_epilogue(ps3, qT0[:, 3, tok_sl0], 0)

        for pc in range(1, PCH // 2):       # region 1: batch 0, k/v/q
            proj_pc(pc, with_q=True)

        pp1_cm.__exit__(None, None, None)

        # ------- regions 2+3: batch-1 projections x attention -------
        with (
            tc.tile_pool(name="psS", bufs=2, space="PSUM") as psS,
            tc.tile_pool(name="psO", bufs=2, space="PSUM") as psO,
            tc.tile_pool(name="psM", bufs=1, space="PSUM") as psM,
            tc.tile_pool(name="psQ", bufs=1, space="PSUM") as psQ,
            tc.tile_pool(name="qbp", bufs=4) as qbp,
            tc.tile_pool(name="expp", bufs=15) as expp,
            tc.tile_pool(name="trep", bufs=9) as trep,
            tc.tile_pool(name="fin", bufs=2) as finp,
        ):
            # pending q-projection work items, injected between score matmuls
            feed_items = []
            fed = [0]

            def feeder(n):
                for _ in range(min(n, len(feed_items))):
                    feed_items.pop(0)()
                    fed[0] += 1

            def flush_to(mark):
                feeder(max(0, mark - fed[0]))

            def qmark():
                return fed[0] + len(feed_items)

            slab_cache = {}

            def queue_qpass(pc, h):
                """Queue one q-projection pass (32 matmuls + rope epilogue)."""
                if h == 0:
                    slab_cache[pc] = load_slab(pc)
                slab = slab_cache[pc]
                ps = psQ.tile([128, PCW], F32, tag="q", name="qps")
                qb = qbp.tile([128, PCW], BF16, tag="qb", name="qb")
                for kc in range(KCH):
                    def mm(kc=kc, ps=ps, slab=slab, h=h):
                        nc.tensor.matmul(
                            ps, wqb[:, kc, h * HD:(h + 1) * HD], slab[kc],
                            start=(kc == 0), stop=(kc == KCH - 1))
                    feed_items.append(mm)

                def epi(ps=ps, qb=qb, pc=pc):
                    rope_epilogue(ps, qb, (pc - PCH // 2) * PCW)
                feed_items.append(epi)
                return qb

            def attn_scores(b, h, sc, q_rhs, W, qoff):
                """scores -> exp, with q-pass matmuls injected between."""
                exps = []
                for g in range(NTT // 2):
                    pS = psS.tile([128, 2 * W], F32, tag="S", name="pS")
                    for j in range(2):
                        tt = 2 * g + j
                        nc.tensor.matmul(
                            pS[:, j * W:(j + 1) * W],
                            kTb[:, b * S + tt * 128:b * S + (tt + 1) * 128],
                            q_rhs, start=True, stop=True)
                    feeder(3 if g == 0 else 2)   # 17 items per chunk
                    eS = expp.tile([128, 2 * W], BF16, tag="e", name="eS")
                    nc.scalar.activation(
                        out=eS, in_=pS,
                        func=mybir.ActivationFunctionType.Exp,
                        scale=SCALE)
                    exps.append(eS)
                return exps

            def attn_av(state):
                b, h, sc, qoff, W, exps = state
                po = psO.tile([128, W], F32, tag="o", name="po")
                for tt in range(NTT):
                    e_rhs = exps[tt // 2][:, (tt % 2) * W:(tt % 2 + 1) * W]
                    nc.tensor.matmul(
                        po, vtok[:, b * NTT + tt, :], e_rhs,
                        start=(tt == 0), stop=(tt == NTT - 1))
                pden = psM.tile([128, W], F32, tag="m", name="pden")
                if pe_tree[0]:
                    # drain chunks: PE is idle, DVE is the critical path --
                    # reduce all 16 exp tiles with ones-matmuls instead
                    for g in range(NTT // 2):
                        for j in range(2):
                            nc.tensor.matmul(
                                pden, ones128, exps[g][:, j * W:(j + 1) * W],
                                start=(g == 0 and j == 0),
                                stop=(g == NTT // 2 - 1 and j == 1))
                else:
                    # denominator: 4-level DVE tree (15 adds) + 1 ones-matmul
                    lvl = []
                    for g in range(NTT // 2):
                        p0 = trep.tile([128, W], BF16, tag="tr0", name="p0")
                        nc.vector.tensor_add(
                            p0, exps[g][:, 0:W], exps[g][:, W:2 * W])
                        lvl.append(p0)
                    tags = {4: ("tr1", 5), 2: ("tr2", 3), 1: ("tr3", 2)}
                    while len(lvl) > 1:
                        tag, bufs = tags[len(lvl) // 2]
                        nxt = []
                        for g in range(len(lvl) // 2):
                            p = trep.tile([128, W], BF16, tag=tag, bufs=bufs)
                            nc.vector.tensor_add(
                                p, lvl[2 * g], lvl[2 * g + 1])
                            nxt.append(p)
                        lvl = nxt
                    nc.tensor.matmul(pden, ones128, lvl[0],
                                     start=True, stop=True)
                recip = finp.tile([128, W], F32, tag="recip", name="recip")
                nc.vector.reciprocal_approx_fast(out=recip, in_=pden)
                return (b, h, sc, qoff, W, po, recip)

            def attn_tail(state):
                b, h, sc, qoff, W, po, recip = state
                osb = finp.tile([128, W], F32, tag="osb", name="osb")
                nc.vector.tensor_mul(osb, po, recip)
                off = sc * SCW + qoff
                nc.sync.dma_start(out=out[b, h, :, off:off + W], in_=osb)

            sc_pend = None
            av_pend = None
            pe_tree = [False]

            def emit_chunk(b, h, sc, q_rhs, W=SCW, qoff=0):
                nonlocal sc_pend, av_pend
                exps = attn_scores(b, h, sc, q_rhs, W, qoff)
                if sc_pend is not None:
                    nxt = attn_av(sc_pend)
                    if av_pend is not None:
                        attn_tail(av_pend)
                    av_pend = nxt
                sc_pend = (b, h, sc, qoff, W, exps)

            def emit_b0(i):
                bh, bsc = divmod(i, SCH)
                emit_chunk(0, bh, bsc,
                           qT0[:, bh, bass.ds(bsc * SCW, SCW)])

            # region 2: batch-1 k/v projection passes block-interleaved with
            # one b0 chunk each, sharing the attention psum tags. The kv pass
            # gives ACT a large head start on exp, so block (not matmul-level)
            # interleave suffices. The chunk comes FIRST in each group so PE
            # has work while the group's x slab streams in; each pc's vtok
            # transposes are deferred past the next chunk so the v-copy has
            # finished by the time PE reaches them.
            def make_transposes(pc):
                def go():
                    pt = psQ.tile([128, PCW], BF16, tag="q", name="pt")
                    for j in range(4):
                        nc.tensor.transpose(
                            pt[:, j * 128:(j + 1) * 128],
                            vTb[:, pc * PCW + j * 128:
                                 pc * PCW + (j + 1) * 128],
                            id_bf)
                    for j in range(4):
                        nc.vector.tensor_copy(vtok[:, pc * 4 + j, :],
                                              pt[:, j * 128:(j + 1) * 128])
                return go

            pending_vt = None
            for pc in range(PCH // 2, PCH):
                emit_b0(pc - PCH // 2)
                if pending_vt is not None:
                    pending_vt()
                slab = load_slab(pc)
                tok_sl = bass.ds(pc * PCW, PCW)
                cc_off = (pc - PCH // 2) * PCW
                psk = psQ.tile([128, PCW], F32, tag="q", name="kps")
                for kc in range(KCH):
                    nc.tensor.matmul(psk, wkb[:, kc, :], slab[kc],
                                     start=(kc == 0), stop=(kc == KCH - 1))
                rope_epilogue(psk, kTb[:, tok_sl], cc_off)
                psv = psM.tile([128, PCW], F32, tag="m", name="vps")
                for kc in range(KCH):
                    nc.tensor.matmul(psv, wvb[:, kc, :], slab[kc],
                                     start=(kc == 0), stop=(kc == KCH - 1))
                nc.scalar.copy(vTb[:, tok_sl], psv)
                pending_vt = make_transposes(pc)
            pending_vt()

            # region 3: per group g (pc,h): queue q-pass g, then emit the
            # next b0 chunk and the b1 chunk of group g-1 (whose q-pass
            # epilogue is guaranteed emitted via flush_to).
            groups = [(pc, h) for pc in range(PCH // 2, PCH)
                      for h in range(HPC)]
            b1_prev = None
            nb0 = SCH
            for g, (pc, h) in enumerate(groups):
                qb = queue_qpass(pc, h)
                mark = qmark()
                if nb0 < HPC * SCH:
                    emit_b0(nb0)
                    nb0 += 1
                if b1_prev is not None:
                    h1, sc1, qb1, mark1 = b1_prev
                    flush_to(mark1)   # q-pass g-1 fully emitted before use
                    emit_chunk(1, h1, sc1, qb1)
                b1_prev = (h, pc - PCH // 2, qb, mark)
            # final chunk in two half-width pieces to shorten the drain
            h1, sc1, qb1, mark1 = b1_prev
            flush_to(mark1)
            HW2 = SCW // 2
            pe_tree[0] = True
            emit_chunk(1, h1, sc1, qb1[:, 0:HW2], W=HW2, qoff=0)
            emit_chunk(1, h1, sc1, qb1[:, HW2:SCW], W=HW2, qoff=HW2)
            nxt = attn_av(sc_pend)
            attn_tail(av_pend)
            attn_tail(nxt)

        wkv_cm.__exit__(None, None, None)
        ropep_cm.__exit__(None, None, None)
        xsp_cm.__exit__(None, None, None)
        wq_cm.__exit__(None, None, None)
        pers_cm.__exit__(None, None, None)

    nc.finalize()
    return nc


_ROPE_PERM = np.concatenate(
    [np.arange(0, HD, 2), np.arange(1, HD, 2)])  # even dims then odd dims


def _shard_inputs(x, wq, wk, wv, freqs_cos, freqs_sin):
    BF = ml_dtypes.bfloat16
    x_flat = np.ascontiguousarray(x.astype(np.float32).reshape(TOK, D))
    xT = np.ascontiguousarray(x_flat.T).astype(BF)               # [D, TOK]
    cosT = np.ascontiguousarray(freqs_cos.T.astype(np.float32))  # [64, S]
    sinT = np.ascontiguousarray(freqs_sin.T.astype(np.float32))
    cc = np.ascontiguousarray(np.concatenate([cosT, cosT], axis=0)).astype(BF)
    ssm = np.ascontiguousarray(np.concatenate([-sinT, sinT], axis=0)).astype(BF)

    in_maps = []
    for c in range(NCORES):
        wq_c = np.empty((D, QDIM), np.float32)
        for j in range(HPC):
            h = HPC * c + j
            wq_c[:, j * HD:(j + 1) * HD] = wq[:, h * HD + _ROPE_PERM]
        wk_c = np.ascontiguousarray(wk[:, c * HD + _ROPE_PERM])
        wv_c = np.ascontiguousarray(wv[:, c * HD:(c + 1) * HD])
        in_maps.append({
            "xt": xT,
            "wq": wq_c.astype(BF), "wk": wk_c.astype(BF),
            "wv": wv_c.astype(BF),
            "cc": cc, "ss": ssm,
        })
    return in_maps


def kernel(x, wq, wk, wv, cache_k, cache_v, freqs_cos, freqs_sin, start_pos):
    global LAST_EXEC_NS
    x = np.asarray(x)
    wq, wk, wv = np.asarray(wq), np.asarray(wk), np.asarray(wv)
    freqs_cos, freqs_sin = np.asarray(freqs_cos), np.asarray(freqs_sin)
    assert int(start_pos) == 0, "kernel specialized for start_pos == 0"
    assert x.shape == (B, S, D)

    nc = _build_program()
    in_maps = _shard_inputs(x, wq, wk, wv, freqs_cos, freqs_sin)
    res = run_bass_kernel_spmd(nc, in_maps, core_ids=list(range(NCORES)))
    LAST_EXEC_NS = res.exec_time_ns

    full = np.empty((B, S, HQ * HD), np.float32)
    for c in range(NCORES):
        o = np.asarray(res.results[c]["out"])      # [B, HPC, HD, S]
        full[:, :, c * QDIM:(c + 1) * QDIM] = (
            o.transpose(0, 3, 1, 2).reshape(B, S, QDIM))
    return full


# revision 28
# speedup vs baseline: 1.0248x; 1.0248x over previous
"""GQA attention block (QKV proj + RoPE + KV cache append + softmax attention)
on 8 Trainium2 NeuronCores, tensor-parallel over heads.

Sharding: core c owns q-heads [4c, 4c+4) and kv-head c. Each core computes its
head slice over all tokens; host concatenates the per-core output columns.

start_pos is specialized to 0 (the cache is zero-filled and fully overwritten
by the current 2048 tokens, so keys/values == rope(x@wk), x@wv).

Schedule (v2): softmax exp() runs on the ACT engine at 1 elem/cycle/partition
and totals ~286us/core -- more than the attention-phase PE work -- so
attention chunks are interleaved with the batch-1 q-projection passes at
matmul granularity, hiding the exp under projection matmuls:
  region 1: k/v/q projections for batch-0 tokens (pc 0..3), rope epilogues
  region 2: k/v projections for batch-1 tokens (pc 4..7)
  region 3: 16 groups of [q-pass (pc,h)] x [2 attention chunks], with the
            q matmuls injected between score matmuls so PE keeps working
            while ACT drains the exp queue.
Output is written untransposed as [B, HPC, HD, S]; the host reassembles.
"""

import sys

sys.path.insert(0, "/opt/trn_rl_repo")

import ml_dtypes
import numpy as np

import concourse.bass as bass
import concourse.tile as tile
from concourse import bacc, mybir
from concourse.bass_utils import run_bass_kernel_spmd
from concourse.masks import make_identity

F32 = mybir.dt.float32
BF16 = mybir.dt.bfloat16

B, S, D = 2, 2048, 4096
HQ, HKV, HD = 32, 8, 128
NCORES = 8
HPC = HQ // NCORES          # q heads per core (4)
QDIM = HPC * HD             # per-core q output dim (512)
TOK = B * S                 # 4096 tokens across both batches
KCH = D // 128              # 32 contraction chunks of 128
PCH = 8                     # projection token chunks
PCW = TOK // PCH            # 512 tokens per chunk
SCH = 4                     # s-chunks per batch in attention
SCW = S // SCH              # 512
NTT = S // 128              # 16 key tiles per batch
SCALE = 1.0 / float(np.sqrt(HD))

LAST_EXEC_NS = None


def _build_program():
    nc = bacc.Bacc("TRN2", target_bir_lowering=False, debug=False,
                   num_devices=NCORES)

    xt = nc.declare_dram_parameter("xt", [D, TOK], BF16, isOutput=False)
    wq = nc.declare_dram_parameter("wq", [D, QDIM], BF16, isOutput=False)
    wk = nc.declare_dram_parameter("wk", [D, HD], BF16, isOutput=False)
    wv = nc.declare_dram_parameter("wv", [D, HD], BF16, isOutput=False)
    cc = nc.declare_dram_parameter("cc", [128, S], BF16, isOutput=False)
    ss = nc.declare_dram_parameter("ss", [128, S], BF16, isOutput=False)
    out = nc.declare_dram_parameter("out", [B, HPC, HD, S], F32,
                                    isOutput=True)

    with tile.TileContext(nc) as tc:
        pers_cm = tc.tile_pool(name="pers", bufs=1)
        pers = pers_cm.__enter__()

        ccs = pers.tile([128, S], BF16)
        sss = pers.tile([128, S], BF16)
        qT0 = pers.tile([128, HPC, S], BF16)     # batch-0 q, [d, head, tok]
        kTb = pers.tile([128, TOK], BF16)        # [d, tok]
        vtok = pers.tile([128, B * NTT, HD], BF16)  # [t, (b,tt), dv]
        ones128 = pers.tile([128, 128], BF16)

        # pool stack (LIFO close order): pers, wqp, xsp, ropep live through
        # region 3; wkv + pp1 close after region 2.
        wq_cm = tc.tile_pool(name="wqp", bufs=1)
        wqp = wq_cm.__enter__()
        xsp_cm = tc.tile_pool(name="xsp", bufs=36)
        xsp = xsp_cm.__enter__()
        ropep_cm = tc.tile_pool(name="ropep", bufs=2)
        ropep = ropep_cm.__enter__()
        wkv_cm = tc.tile_pool(name="wkv", bufs=1)
        wkv = wkv_cm.__enter__()

        wkb = wkv.tile([128, KCH, HD], BF16)
        wvb = wkv.tile([128, KCH, HD], BF16)
        vTb = wkv.tile([128, TOK], BF16)         # [dv, tok], regions 1-2
        id_bf = wkv.tile([128, 128], BF16)
        wqb = wqp.tile([128, KCH, QDIM], BF16)
        # weights all on the gpsimd queue in need-order (k+v pass first,
        # then q passes); the sync queue carries x exclusively so the
        # startup HBM demand stays under the per-core bandwidth.
        for kc in range(KCH):
            nc.gpsimd.dma_start(out=wkb[:, kc, :],
                                in_=wk[kc * 128:(kc + 1) * 128, :])
        for kc in range(KCH):
            nc.gpsimd.dma_start(out=wvb[:, kc, :],
                                in_=wv[kc * 128:(kc + 1) * 128, :])
        for kc in range(KCH):
            nc.gpsimd.dma_start(out=wqb[:, kc, :],
                                in_=wq[kc * 128:(kc + 1) * 128, :])
        # cc/ss last: first needed by pc0's k-epilogue (~55us in)
        nc.gpsimd.dma_start(out=ccs, in_=cc[:])
        nc.gpsimd.dma_start(out=sss, in_=ss[:])
        make_identity(nc, id_bf)
        nc.vector.memset(ones128, 1.0)

        def load_slab(pc):
            tiles = []
            for kc in range(KCH):
                xf = xsp.tile([128, PCW], BF16, tag="xs", name="xs")
                nc.sync.dma_start(
                    out=xf,
                    in_=xt[kc * 128:(kc + 1) * 128,
                           pc * PCW:(pc + 1) * PCW])
                tiles.append(xf)
            return tiles

        def rope_epilogue(ps, dst, cc_off):
            cc_sl = bass.ds(cc_off, PCW)
            t1 = ropep.tile([128, PCW], F32, tag="t1")
            t2 = ropep.tile([128, PCW], F32, tag="t2")
            swp = ropep.tile([128, PCW], F32, tag="swp")
            nc.vector.tensor_mul(t1, ps, ccs[:, cc_sl])
            # pair-partner swap: cross-partition-base copies
            nc.scalar.copy(swp[0:64], ps[64:128])
            nc.scalar.copy(swp[64:128], ps[0:64])
            nc.vector.tensor_mul(t2, swp, sss[:, cc_sl])
            nc.vector.tensor_add(dst, t1, t2)

        # ---------------- regions 1+2: projections ----------------
        # kc-major: each x tile is consumed by its 6 (or 2) matmuls
        # back-to-back, so its ring slot frees ~1.3us after the DMA and the
        # next slab streams in fully overlapped.
        pp1_cm = tc.tile_pool(name="pp1", bufs=6, space="PSUM")
        pp1 = pp1_cm.__enter__()

        def v_transposes_pc(pc):
            # 4 token blocks of this pc, spread per-pc so the PE/DVE work
            # never bunches at a region boundary
            for j in range(4):
                tt = pc * 4 + j
                pt = pp1.tile([128, 128], BF16, tag="vt", name="pt", bufs=2)
                nc.tensor.transpose(
                    pt, vTb[:, tt * 128:(tt + 1) * 128], id_bf)
                nc.vector.tensor_copy(vtok[:, tt, :], pt)

        def proj_pc(pc, with_q):
            tok_sl = bass.ds(pc * PCW, PCW)
            cc_off = (pc * PCW) % S
            nps = 2 + (HPC if with_q else 0)
            lhs = [wkb, wvb] + [wqb[:, :, h * HD:(h + 1) * HD]
                                for h in range(HPC)][:nps - 2]
            psums = [pp1.tile([128, PCW], F32, tag="proj", name="proj")
                     for _ in range(nps)]
            for kc in range(KCH):
                xf = xsp.tile([128, PCW], BF16, tag="xs", name="xs")
                nc.sync.dma_start(
                    out=xf, in_=xt[kc * 128:(kc + 1) * 128,
                                   pc * PCW:(pc + 1) * PCW])
                for ot in range(nps):
                    nc.tensor.matmul(psums[ot], lhs[ot][:, kc, :], xf,
                                     start=(kc == 0), stop=(kc == KCH - 1))
            rope_epilogue(psums[0], kTb[:, tok_sl], cc_off)
            nc.scalar.copy(vTb[:, tok_sl], psums[1])
            v_transposes_pc(pc)
            for h in range(nps - 2):
                rope_epilogue(psums[2 + h],
                              qT0[:, h, bass.ds(pc * PCW, PCW)], cc_off)

        # region 1: batch 0, k/v/q. pc 0..2 kc-major. pc 3 runs as three
        # PAIRED passes [k+v][q0+q1][q2+q3] with per-pair epilogues so the
        # proj psum banks 0..3 (which the attention pS pool aliases) are
        # already free when the first attention chunk starts.
        for pc in range(0, PCH // 2 - 1):
            proj_pc(pc, with_q=True)

        pc3 = PCH // 2 - 1
        slab3 = load_slab(pc3)
        tok_sl3 = bass.ds(pc3 * PCW, PCW)

        def pass3_pair(lhsA, lhsB):
            psA = pp1.tile([128, PCW], F32, tag="proj", name="proj")
            psB = pp1.tile([128, PCW], F32, tag="proj", name="proj")
            for kc in range(KCH):
                nc.tensor.matmul(psA, lhsA[:, kc, :], slab3[kc],
                                 start=(kc == 0), stop=(kc == KCH - 1))
                nc.tensor.matmul(psB, lhsB[:, kc, :], slab3[kc],
                                 start=(kc == 0), stop=(kc == KCH - 1))
            return psA, psB

        psk3, psv3 = pass3_pair(wkb, wvb)
        rope_epilogue(psk3, kTb[:, tok_sl3], pc3 * PCW)
        nc.scalar.copy(vTb[:, tok_sl3], psv3)
        ps0, ps1 = pass3_pair(wqb[:, :, 0 * HD:1 * HD],
                              wqb[:, :, 1 * HD:2 * HD])
        rope_epilogue(ps0, qT0[:, 0, tok_sl3], pc3 * PCW)
        rope_epilogue(ps1, qT0[:, 1, tok_sl3], pc3 * PCW)
        v_transposes_pc(pc3)
        ps2, ps3b = pass3_pair(wqb[:, :, 2 * HD:3 * HD],
                               wqb[:, :, 3 * HD:4 * HD])
        rope_epilogue(ps2, qT0[:, 2, tok_sl3], pc3 * PCW)
        rope_epilogue(ps3b, qT0[:, 3, tok_sl3], pc3 * PCW)

        pp1_cm.__exit__(None, None, None)

        # ------- regions 2+3: batch-1 projections x attention -------
        with (
            tc.tile_pool(name="psS", bufs=2, space="PSUM") as psS,
            tc.tile_pool(name="psO", bufs=2, space="PSUM") as psO,
            tc.tile_pool(name="psM", bufs=1, space="PSUM") as psM,
            tc.tile_pool(name="psQ", bufs=1, space="PSUM") as psQ,
            tc.tile_pool(name="qbp", bufs=4) as qbp,
            tc.tile_pool(name="expp", bufs=15) as expp,
            tc.tile_pool(name="trep", bufs=9) as trep,
            tc.tile_pool(name="fin", bufs=2) as finp,
        ):
            # pending q-projection work items, injected between score matmuls
            feed_items = []
            fed = [0]

            def feeder(n):
                for _ in range(min(n, len(feed_items))):
                    feed_items.pop(0)()
                    fed[0] += 1

            def flush_to(mark):
                feeder(max(0, mark - fed[0]))

            def qmark():
                return fed[0] + len(feed_items)

            slab_cache = {}

            def queue_qpass(pc, h):
                """Queue one q-projection pass (32 matmuls + rope epilogue)."""
                if h == 0:
                    slab_cache[pc] = load_slab(pc)
                slab = slab_cache[pc]
                ps = psQ.tile([128, PCW], F32, tag="q", name="qps")
                qb = qbp.tile([128, PCW], BF16, tag="qb", name="qb")
                for kc in range(KCH):
                    def mm(kc=kc, ps=ps, slab=slab, h=h):
                        nc.tensor.matmul(
                            ps, wqb[:, kc, h * HD:(h + 1) * HD], slab[kc],
                            start=(kc == 0), stop=(kc == KCH - 1))
                    feed_items.append(mm)

                def epi(ps=ps, qb=qb, pc=pc):
                    rope_epilogue(ps, qb, (pc - PCH // 2) * PCW)
                feed_items.append(epi)
                return qb

            def attn_scores(b, h, sc, q_rhs, W, qoff):
                """scores -> exp, with q-pass matmuls injected between."""
                exps = []
                for g in range(NTT // 2):
                    pS = psS.tile([128, 2 * W], F32, tag="S", name="pS")
                    for j in range(2):
                        tt = 2 * g + j
                        nc.tensor.matmul(
                            pS[:, j * W:(j + 1) * W],
                            kTb[:, b * S + tt * 128:b * S + (tt + 1) * 128],
                            q_rhs, start=True, stop=True)
                    feeder(3 if g == 0 else 2)   # 17 items per chunk
                    eS = expp.tile([128, 2 * W], BF16, tag="e", name="eS")
                    nc.scalar.activation(
                        out=eS, in_=pS,
                        func=mybir.ActivationFunctionType.Exp,
                        scale=SCALE)
                    exps.append(eS)
                return exps

            def attn_av(state):
                b, h, sc, qoff, W, exps = state
                po = psO.tile([128, W], F32, tag="o", name="po")
                for tt in range(NTT):
                    e_rhs = exps[tt // 2][:, (tt % 2) * W:(tt % 2 + 1) * W]
                    nc.tensor.matmul(
                        po, vtok[:, b * NTT + tt, :], e_rhs,
                        start=(tt == 0), stop=(tt == NTT - 1))
                pden = psM.tile([128, W], F32, tag="m", name="pden")
                if pe_tree[0]:
                    # drain chunks: PE is idle, DVE is the critical path --
                    # reduce all 16 exp tiles with ones-matmuls instead
                    for g in range(NTT // 2):
                        for j in range(2):
                            nc.tensor.matmul(
                                pden, ones128, exps[g][:, j * W:(j + 1) * W],
                                start=(g == 0 and j == 0),
                                stop=(g == NTT // 2 - 1 and j == 1))
                else:
                    # denominator: 4-level DVE tree (15 adds) + 1 ones-matmul
                    lvl = []
                    for g in range(NTT // 2):
                        p0 = trep.tile([128, W], BF16, tag="tr0", name="p0")
                        nc.vector.tensor_add(
                            p0, exps[g][:, 0:W], exps[g][:, W:2 * W])
                        lvl.append(p0)
                    tags = {4: ("tr1", 5), 2: ("tr2", 3), 1: ("tr3", 2)}
                    while len(lvl) > 1:
                        tag, bufs = tags[len(lvl) // 2]
                        nxt = []
                        for g in range(len(lvl) // 2):
                            p = trep.tile([128, W], BF16, tag=tag, bufs=bufs)
                            nc.vector.tensor_add(
                                p, lvl[2 * g], lvl[2 * g + 1])
                            nxt.append(p)
                        lvl = nxt
                    nc.tensor.matmul(pden, ones128, lvl[0],
                                     start=True, stop=True)
                recip = finp.tile([128, W], F32, tag="recip", name="recip")
                nc.vector.reciprocal_approx_fast(out=recip, in_=pden)
                return (b, h, sc, qoff, W, po, recip)

            def attn_tail(state):
                b, h, sc, qoff, W, po, recip = state
                osb = finp.tile([128, W], F32, tag="osb", name="osb")
                nc.vector.tensor_mul(osb, po, recip)
                off = sc * SCW + qoff
                nc.sync.dma_start(out=out[b, h, :, off:off + W], in_=osb)

            sc_pend = None
            av_pend = None
            pe_tree = [False]

            def emit_chunk(b, h, sc, q_rhs, W=SCW, qoff=0):
                nonlocal sc_pend, av_pend
                exps = attn_scores(b, h, sc, q_rhs, W, qoff)
                if sc_pend is not None:
                    nxt = attn_av(sc_pend)
                    if av_pend is not None:
                        attn_tail(av_pend)
                    av_pend = nxt
                sc_pend = (b, h, sc, qoff, W, exps)

            def emit_b0(i):
                bh, bsc = divmod(i, SCH)
                emit_chunk(0, bh, bsc,
                           qT0[:, bh, bass.ds(bsc * SCW, SCW)])

            # region 2: batch-1 k/v projection passes block-interleaved with
            # one b0 chunk each, sharing the attention psum tags. The kv pass
            # gives ACT a large head start on exp, so block (not matmul-level)
            # interleave suffices. The chunk comes FIRST in each group so PE
            # has work while the group's x slab streams in; each pc's vtok
            # transposes are deferred past the next chunk so the v-copy has
            # finished by the time PE reaches them.
            def make_transposes(pc):
                def go():
                    pt = psQ.tile([128, PCW], BF16, tag="q", name="pt")
                    for j in range(4):
                        nc.tensor.transpose(
                            pt[:, j * 128:(j + 1) * 128],
                            vTb[:, pc * PCW + j * 128:
                                 pc * PCW + (j + 1) * 128],
                            id_bf)
                    for j in range(4):
                        nc.vector.tensor_copy(vtok[:, pc * 4 + j, :],
                                              pt[:, j * 128:(j + 1) * 128])
                return go

            pending_vt = None
            for pc in range(PCH // 2, PCH):
                emit_b0(pc - PCH // 2)
                if pending_vt is not None:
                    pending_vt()
                slab = load_slab(pc)
                tok_sl = bass.ds(pc * PCW, PCW)
                cc_off = (pc - PCH // 2) * PCW
                psk = psQ.tile([128, PCW], F32, tag="q", name="kps")
                for kc in range(KCH):
                    nc.tensor.matmul(psk, wkb[:, kc, :], slab[kc],
                                     start=(kc == 0), stop=(kc == KCH - 1))
                rope_epilogue(psk, kTb[:, tok_sl], cc_off)
                psv = psM.tile([128, PCW], F32, tag="m", name="vps")
                for kc in range(KCH):
                    nc.tensor.matmul(psv, wvb[:, kc, :], slab[kc],
                                     start=(kc == 0), stop=(kc == KCH - 1))
                nc.scalar.copy(vTb[:, tok_sl], psv)
                pending_vt = make_transposes(pc)
            pending_vt()

            # region 3: per group g (pc,h): queue q-pass g, then emit the
            # next b0 chunk and the b1 chunk of group g-1 (whose q-pass
            # epilogue is guaranteed emitted via flush_to).
            groups = [(pc, h) for pc in range(PCH // 2, PCH)
                      for h in range(HPC)]
            b1_prev = None
            nb0 = SCH
            for g, (pc, h) in enumerate(groups):
                qb = queue_qpass(pc, h)
                mark = qmark()
                if nb0 < HPC * SCH:
                    emit_b0(nb0)
                    nb0 += 1
                if b1_prev is not None:
                    h1, sc1, qb1, mark1 = b1_prev
                    flush_to(mark1)   # q-pass g-1 fully emitted before use
                    emit_chunk(1, h1, sc1, qb1)
                b1_prev = (h, pc - PCH // 2, qb, mark)
            # final chunk in two half-width pieces to shorten the drain
            h1, sc1, qb1, mark1 = b1_prev
            flush_to(mark1)
            HW2 = SCW // 2
            pe_tree[0] = True
            emit_chunk(1, h1, sc1, qb1[:, 0:HW2], W=HW2, qoff=0)
            emit_chunk(1, h1, sc1, qb1[:, HW2:SCW], W=HW2, qoff=HW2)
            nxt = attn_av(sc_pend)
            attn_tail(av_pend)
            attn_tail(nxt)

        wkv_cm.__exit__(None, None, None)
        ropep_cm.__exit__(None, None, None)
        xsp_cm.__exit__(None, None, None)
        wq_cm.__exit__(None, None, None)
        pers_cm.__exit__(None, None, None)

    nc.finalize()
    return nc


_ROPE_PERM = np.concatenate(
    [np.arange(0, HD, 2), np.arange(1, HD, 2)])  # even dims then odd dims


def _shard_inputs(x, wq, wk, wv, freqs_cos, freqs_sin):
    BF = ml_dtypes.bfloat16
    x_flat = np.ascontiguousarray(x.astype(np.float32).reshape(TOK, D))
    xT = np.ascontiguousarray(x_flat.T).astype(BF)               # [D, TOK]
    cosT = np.ascontiguousarray(freqs_cos.T.astype(np.float32))  # [64, S]
    sinT = np.ascontiguousarray(freqs_sin.T.astype(np.float32))
    cc = np.ascontiguousarray(np.concatenate([cosT, cosT], axis=0)).astype(BF)
    ssm = np.ascontiguousarray(np.concatenate([-sinT, sinT], axis=0)).astype(BF)

    in_maps = []
    for c in range(NCORES):
        wq_c = np.empty((D, QDIM), np.float32)
        for j in range(HPC):
            h = HPC * c + j
            wq_c[:, j * HD:(j + 1) * HD] = wq[:, h * HD + _ROPE_PERM]
        wk_c = np.ascontiguousarray(wk[:, c * HD + _ROPE_PERM])
        wv_c = np.ascontiguousarray(wv[:, c * HD:(c + 1) * HD])
        in_maps.append({
            "xt": xT,
            "wq": wq_c.astype(BF), "wk": wk_c.astype(BF),
            "wv": wv_c.astype(BF),
            "cc": cc, "ss": ssm,
        })
    return in_maps


def kernel(x, wq, wk, wv, cache_k, cache_v, freqs_cos, freqs_sin, start_pos):
    global LAST_EXEC_NS
    x = np.asarray(x)
    wq, wk, wv = np.asarray(wq), np.asarray(wk), np.asarray(wv)
    freqs_cos, freqs_sin = np.asarray(freqs_cos), np.asarray(freqs_sin)
    assert int(start_pos) == 0, "kernel specialized for start_pos == 0"
    assert x.shape == (B, S, D)

    nc = _build_program()
    in_maps = _shard_inputs(x, wq, wk, wv, freqs_cos, freqs_sin)
    res = run_bass_kernel_spmd(nc, in_maps, core_ids=list(range(NCORES)))
    LAST_EXEC_NS = res.exec_time_ns

    full = np.empty((B, S, HQ * HD), np.float32)
    for c in range(NCORES):
        o = np.asarray(res.results[c]["out"])      # [B, HPC, HD, S]
        full[:, :, c * QDIM:(c + 1) * QDIM] = (
            o.transpose(0, 3, 1, 2).reshape(B, S, QDIM))
    return full
